# revision 1
# baseline (speedup 1.0000x reference)
"""Trainium2 Bass kernel for nn_ContextualViewModel_48833778155979.

Computation (see reference):
    station_feats = x[sx, sy]            # (K, F) gather -- done on host (hint: replicate)
    y = station_feats @ W                # (K, F) tiny matmul -- on device, fp32
    res[h, w, :] = sum_k d[h, w, k] * y[k, :]   # big (H*W, K) @ (K, F) matmul -- on device

Sharding: H axis split across 8 cores (48 rows each -> 18432 grid cells/core).
Per core the big matmul is (18432, 256) @ (256, 256) fp32.

Device strategy per core:
  - d streamed in 1 MiB slabs (1024 rows) as (128p, 8, 256) tiles; each
    128-row subtile is PE-transposed (exact) to get the k-major stationary
    operand, then two float32r matmuls (full-rate fp32 path, N=256)
    accumulate over the two 128-wide k chunks into PSUM; result staged to an
    SBUF slab and stored with one 1 MiB DMA.
  - y = (x[sx,sy]) @ W computed once on device with regular (precise) fp32
    matmuls from a host-provided transposed gather.
"""

import sys

sys.path.insert(0, "/opt/trn_rl_repo")

from contextlib import ExitStack

import numpy as np

import concourse.bacc as bacc
import concourse.mybir as mybir
import concourse.tile as tile
from concourse.bass_utils import run_bass_kernel_spmd

H, WG, F = 384, 384, 256
K = 256
NCORES = 8
HS = H // NCORES          # 48 grid rows per core
ROWS = HS * WG            # 18432 cells per core
SLAB = 1024               # rows per DMA slab (1 MiB fp32)
NSLAB = ROWS // SLAB      # 18
NSUB = SLAB // 128        # 8 subtiles of 128 rows per slab

F32 = mybir.dt.float32
F32R = mybir.dt.float32r

_cache: dict = {}
last_results = None  # BassKernelResults of the most recent kernel() call


def _build_program(reps: int = 1):
    key = ("nc", reps)
    if key in _cache:
        return _cache[key]

    nc = bacc.Bacc(
        "TRN2", target_bir_lowering=False, debug=False, num_devices=NCORES
    )

    d_ext = nc.dram_tensor("d_shard", [ROWS, K], F32, kind="ExternalInput").ap()
    stT_ext = nc.dram_tensor("station_t", [K, K], F32, kind="ExternalInput").ap()
    w_ext = nc.dram_tensor("w_mat", [K, F], F32, kind="ExternalInput").ap()
    id_ext = nc.dram_tensor("ident", [128, 128], F32, kind="ExternalInput").ap()
    out_ext = nc.dram_tensor("out_shard", [ROWS, F], F32, kind="ExternalOutput").ap()

    with tile.TileContext(nc) as tc, ExitStack() as ctx:
        const = ctx.enter_context(tc.tile_pool(name="const", bufs=1))
        dpool = ctx.enter_context(tc.tile_pool(name="din", bufs=3))
        opool = ctx.enter_context(tc.tile_pool(name="dout", bufs=3))
        dtpool = ctx.enter_context(tc.tile_pool(name="dt", bufs=3))
        tpsum = ctx.enter_context(tc.tile_pool(name="tpsum", bufs=2, space="PSUM"))
        mpsum = ctx.enter_context(tc.tile_pool(name="mpsum", bufs=2, space="PSUM"))
        ypsum = ctx.enter_context(tc.tile_pool(name="ypsum", bufs=1, space="PSUM"))

        # --- constants -----------------------------------------------------
        ident = const.tile([128, 128], F32)
        nc.sync.dma_start(ident[:, :], id_ext)

        # station_T (c, k): chunk the contraction dim c into 2x128
        stT = const.tile([128, 2, K], F32)
        nc.sync.dma_start(
            stT[:, :, :], stT_ext.rearrange("(cc cp) k -> cp cc k", cc=2)
        )
        w_sb = const.tile([128, 2, F], F32)
        nc.sync.dma_start(
            w_sb[:, :, :], w_ext.rearrange("(cc cp) f -> cp cc f", cc=2)
        )

        # --- y = station_feats @ W  (precise fp32), y[k, f] k-major --------
        # y_sb is float32r: the copy out of PSUM rounds it for the fp32r
        # matmuls below (walrus requires fp32r operands to be produced as
        # fp32r).
        y_sb = const.tile([128, 2, F], F32R)
        for kc in range(2):
            yps = ypsum.tile([128, F], F32, tag="ypsum")
            for cc in range(2):
                nc.tensor.matmul(
                    yps[:, :],
                    stT[:, cc, kc * 128 : (kc + 1) * 128],
                    w_sb[:, cc, :],
                    start=(cc == 0),
                    stop=(cc == 1),
                )
            nc.vector.tensor_copy(y_sb[:, kc, :], yps[:, :])

        # --- main loop: out = d @ y ---------------------------------------
        # reps > 1 wraps the identical (idempotent) pipeline in a hardware
        # loop so a benchmark can difference wall times to isolate device
        # exec time. The graded path is reps=1: no loop machinery.
        def emit_pipeline():
            for s in range(NSLAB):
                emit_slab(s)

        def emit_slab(s):
            din = dpool.tile([128, NSUB, K], F32, tag="din")
            nc.sync.dma_start(
                din[:, :, :],
                d_ext[s * SLAB : (s + 1) * SLAB, :].rearrange(
                    "(p n) k -> p n k", n=NSUB
                ),
            )  # noqa: E501
            dout = opool.tile([128, NSUB, F], F32, tag="dout")
            for n in range(NSUB):
                ptA = tpsum.tile([128, 128], F32, tag="ptA")
                ptB = tpsum.tile([128, 128], F32, tag="ptB")
                nc.tensor.transpose(ptA[:, :], din[:, n, 0:128], ident[:, :])
                nc.tensor.transpose(ptB[:, :], din[:, n, 128:256], ident[:, :])
                dTa = dtpool.tile([128, 128], F32R, tag="dTa")
                dTb = dtpool.tile([128, 128], F32R, tag="dTb")
                nc.scalar.copy(dTa[:, :], ptA[:, :])
                nc.scalar.copy(dTb[:, :], ptB[:, :])
                po = mpsum.tile([128, F], F32, tag="po")
                nc.tensor.matmul(
                    po[:, :],
                    dTa[:, :],
                    y_sb[:, 0, :],
                    start=True,
                    stop=False,
                )
                nc.tensor.matmul(
                    po[:, :],
                    dTb[:, :],
                    y_sb[:, 1, :],
                    start=False,
                    stop=True,
                )
                nc.vector.tensor_copy(dout[:, n, :], po[:, :])
            nc.scalar.dma_start(
                out_ext[s * SLAB : (s + 1) * SLAB, :].rearrange(
                    "(p n) f -> p n f", n=NSUB
                ),
                dout[:, :, :],
            )

        if reps == 1:
            emit_pipeline()
        else:
            with tc.For_i(0, reps, 1):
                emit_pipeline()

    nc.compile()
    _cache[key] = nc
    return nc


def kernel(x, d, W, sx, sy):
    x = np.asarray(x, dtype=np.float32)
    d = np.asarray(d, dtype=np.float32)
    W = np.asarray(W, dtype=np.float32)
    sx = np.asarray(sx, dtype=np.int32)
    sy = np.asarray(sy, dtype=np.int32)

    # Host-side gather of the K station feature vectors (replicated to all
    # cores, per the sharding strategy), pre-transposed to contraction-major.
    station_t = np.ascontiguousarray(x[sx, sy].T)
    ident = np.eye(128, dtype=np.float32)

    nc = _build_program()

    in_maps = []
    for c in range(NCORES):
        d_shard = np.ascontiguousarray(
            d[c * HS : (c + 1) * HS].reshape(ROWS, K)
        )
        in_maps.append(
            {
                "d_shard": d_shard,
                "station_t": station_t,
                "w_mat": W,
                "ident": ident,
            }
        )

    res = run_bass_kernel_spmd(nc, in_maps, list(range(NCORES)))
    global last_results
    last_results = res
    out = np.concatenate(
        [r["out_shard"].reshape(HS, WG, F) for r in res.results], axis=0
    )
    return out


if __name__ == "__main__":
    rng = np.random.default_rng(0)
    x = rng.standard_normal((H, WG, F), dtype=np.float32)
    d = rng.random((H, WG, K), dtype=np.float32)
    W = rng.standard_normal((K, F), dtype=np.float32) / np.sqrt(F)
    sx = rng.integers(0, H, size=(K,)).astype(np.int32)
    sy = rng.integers(0, WG, size=(K,)).astype(np.int32)
    out = kernel(x, d, W, sx, sy)
    y = x[sx, sy].astype(np.float64) @ W.astype(np.float64)
    exp = d.reshape(-1, K).astype(np.float64) @ y
    exp = exp.reshape(H, WG, F)
    err = np.linalg.norm(out - exp) / np.linalg.norm(exp)
    print("rel err:", err)



# revision 5
# speedup vs baseline: 1.6518x; 1.6518x over previous
"""Trainium2 Bass kernel for nn_ContextualViewModel_48833778155979.

Computation (see reference):
    station_feats = x[sx, sy]            # (K, F) gather -- on host (replicated)
    y = station_feats @ W                # (K, F) tiny matmul -- on device, fp32
    res[h, w, :] = sum_k d[h, w, k] * y[k, :]   # big (H*W, K) @ (K, F) matmul

Sharding: H axis split across 8 cores (48 rows each -> 18432 grid cells/core).

Device strategy per core (v2 -- transpose-free, bf16 I/O):
  - The host ships d pre-transposed per core as d_t = (K, 18432) bf16, so the
    contraction dim k is already on SBUF partitions: no on-device transposes.
  - y = (x[sx,sy]) @ W is computed once on device with precise fp32 matmuls
    and cast to bf16; its four 128x128 chunks are the stationary operands.
  - Main loop streams d_t as the bf16 moving operand: per 2048-column block,
    16 matmuls of N=512 accumulate over the two 128-wide k chunks into PSUM
    (one full bank each), producing out^T[f, m] in fp32.
  - PSUM -> SBUF copies cast to bf16, split across the vector engine (f-half
    0) and scalar engine (f-half 1); each f-half is stored with its own
    512 KiB DMA on the scalar HWDGE ring while input DMAs ride the sync ring.
  - The host casts the (F, 18432) bf16 shards up to fp32 and transposes back.

bf16 rounding of d, y and out adds ~2e-3 relative error (tolerance 1e-2).
"""

import sys

sys.path.insert(0, "/opt/trn_rl_repo")

from contextlib import ExitStack

import ml_dtypes
import numpy as np

import concourse.bacc as bacc
import concourse.mybir as mybir
import concourse.tile as tile
from concourse.bass_utils import run_bass_kernel_spmd

H, WG, F = 384, 384, 256
K = 256
NCORES = 8
HS = H // NCORES          # 48 grid rows per core
ROWS = HS * WG            # 18432 cells per core
BLK = 2048                # output columns per DMA block (1 MiB bf16 in)
NBLK = ROWS // BLK        # 9
NSB = BLK // 512          # 4 matmul sub-blocks of N=512 per block

F32 = mybir.dt.float32
BF16 = mybir.dt.bfloat16

_cache: dict = {}
last_results = None  # BassKernelResults of the most recent kernel() call


def _build_program():
    if "nc" in _cache:
        return _cache["nc"]

    nc = bacc.Bacc(
        "TRN2", target_bir_lowering=False, debug=False, num_devices=NCORES
    )

    dt_ext = nc.dram_tensor("d_t", [K, ROWS], BF16, kind="ExternalInput").ap()
    stT_ext = nc.dram_tensor("station_t", [K, K], F32, kind="ExternalInput").ap()
    w_ext = nc.dram_tensor("w_mat", [K, F], F32, kind="ExternalInput").ap()
    out_ext = nc.dram_tensor("out_t", [F, ROWS], BF16, kind="ExternalOutput").ap()

    with tile.TileContext(nc) as tc, ExitStack() as ctx:
        const = ctx.enter_context(tc.tile_pool(name="const", bufs=1))
        dpool = ctx.enter_context(tc.tile_pool(name="din", bufs=3))
        o0pool = ctx.enter_context(tc.tile_pool(name="dout0", bufs=3))
        o1pool = ctx.enter_context(tc.tile_pool(name="dout1", bufs=3))
        # 8 tags x 1 buf x [128, 512] f32 = exactly the 8 PSUM banks
        mpsum = ctx.enter_context(tc.tile_pool(name="mpsum", bufs=1, space="PSUM"))

        # --- constants: station_t (c, k) and W (c, f), c chunked 2x128 ------
        stT = const.tile([128, 2, K], F32)
        nc.sync.dma_start(
            stT[:, :, :], stT_ext.rearrange("(cc cp) k -> cp cc k", cc=2)
        )
        w_sb = const.tile([128, 2, F], F32)
        nc.sync.dma_start(
            w_sb[:, :, :], w_ext.rearrange("(cc cp) f -> cp cc f", cc=2)
        )

        # --- y = station_feats @ W (precise fp32), k-major, cast to bf16 ----
        y_sb = const.tile([128, 2, F], BF16)
        for kc in range(2):
            yps = mpsum.tile([128, 512], F32, name=f"yps{kc}", tag=f"po0{kc}")
            for cc in range(2):
                nc.tensor.matmul(
                    yps[:, :F],
                    stT[:, cc, kc * 128 : (kc + 1) * 128],
                    w_sb[:, cc, :],
                    start=(cc == 0),
                    stop=(cc == 1),
                )
            nc.vector.tensor_copy(y_sb[:, kc, :], yps[:, :F])

        # --- main loop: out^T[f, m] = sum_k y[k, f] * d_t[k, m] -------------
        for b in range(NBLK):
            m0 = b * BLK
            din = dpool.tile([128, 2, BLK], BF16, tag="din")
            nc.sync.dma_start(
                din[:, :, :],
                dt_ext[:, m0 : m0 + BLK].rearrange("(kc kp) m -> kp kc m", kc=2),
            )
            for fh in range(2):
                opool = o0pool if fh == 0 else o1pool
                dout = opool.tile([128, BLK], BF16, tag=f"dout{fh}")
                pos = []
                for kc in range(2):
                    for sb in range(NSB):
                        if kc == 0:
                            pos.append(
                                mpsum.tile(
                                    [128, 512],
                                    F32,
                                    name=f"po{fh}{sb}",
                                    tag=f"po{fh}{sb}",
                                )
                            )
                        nc.tensor.matmul(
                            pos[sb][:, :],
                            y_sb[:, kc, fh * 128 : (fh + 1) * 128],
                            din[:, kc, sb * 512 : (sb + 1) * 512],
                            start=(kc == 0),
                            stop=(kc == 1),
                        )
                copy = nc.vector.tensor_copy if fh == 0 else nc.scalar.copy
                for sb in range(NSB):
                    copy(dout[:, sb * 512 : (sb + 1) * 512], pos[sb][:, :])
                nc.scalar.dma_start(
                    out_ext[fh * 128 : (fh + 1) * 128, m0 : m0 + BLK],
                    dout[:, :],
                )

    nc.compile()
    _cache["nc"] = nc
    return nc


def kernel(x, d, W, sx, sy):
    x = np.asarray(x, dtype=np.float32)
    d = np.asarray(d, dtype=np.float32)
    W = np.asarray(W, dtype=np.float32)
    sx = np.asarray(sx, dtype=np.int32)
    sy = np.asarray(sy, dtype=np.int32)

    # Host-side gather of the K station feature vectors (replicated to all
    # cores, per the sharding strategy), pre-transposed to contraction-major.
    station_t = np.ascontiguousarray(x[sx, sy].T)
    bf16 = ml_dtypes.bfloat16

    nc = _build_program()

    dd = d.reshape(NCORES, ROWS, K)
    in_maps = []
    for c in range(NCORES):
        in_maps.append(
            {
                "d_t": dd[c].T.astype(bf16),  # (K, ROWS) contraction-major
                "station_t": station_t,
                "w_mat": W,
            }
        )

    res = run_bass_kernel_spmd(nc, in_maps, list(range(NCORES)))
    global last_results
    last_results = res
    out = np.concatenate(
        [
            np.asarray(r["out_t"]).astype(np.float32).T.reshape(HS, WG, F)
            for r in res.results
        ],
        axis=0,
    )
    return out


if __name__ == "__main__":
    rng = np.random.default_rng(0)
    x = rng.standard_normal((H, WG, F), dtype=np.float32)
    d = rng.random((H, WG, K), dtype=np.float32)
    W = rng.standard_normal((K, F), dtype=np.float32) / np.sqrt(F)
    sx = rng.integers(0, H, size=(K,)).astype(np.int32)
    sy = rng.integers(0, WG, size=(K,)).astype(np.int32)
    out = kernel(x, d, W, sx, sy)
    y = x[sx, sy].astype(np.float64) @ W.astype(np.float64)
    exp = d.reshape(-1, K).astype(np.float64) @ y
    exp = exp.reshape(H, WG, F)
    err = np.linalg.norm(out - exp) / np.linalg.norm(exp)
    print("rel err:", err)


# revision 7
# speedup vs baseline: 1.7305x; 1.0477x over previous
"""Trainium2 Bass kernel for nn_ContextualViewModel_48833778155979.

Computation (see reference):
    station_feats = x[sx, sy]            # (K, F) gather -- on host (replicated)
    y = station_feats @ W                # (K, F) tiny matmul -- on device, fp32
    res[h, w, :] = sum_k d[h, w, k] * y[k, :]   # big (H*W, K) @ (K, F) matmul

Sharding: H axis split across 8 cores (48 rows each -> 18432 grid cells/core).

Device strategy per core (v3 -- transpose-free, bf16 I/O, 3 DMA rings):
  - The host ships d pre-transposed per core as d_t = (K, 18432) bf16, so the
    contraction dim k is already on SBUF partitions: no on-device transposes.
  - y = (x[sx,sy]) @ W is computed once on device with precise fp32 matmuls
    and cast to bf16; its four 128x128 chunks are the stationary operands.
  - Main loop streams d_t as the bf16 moving operand: per 2048-column block,
    16 matmuls of N=512 accumulate over the two 128-wide k chunks into four
    2-bank PSUM tiles, producing out^T[f, m] in fp32.
  - Ring split so nothing head-of-line blocks: input DMAs (two 512 KiB
    per block, one per k chunk) ride the sync HWDGE ring; the two constant
    DMAs ride the scalar HWDGE ring; output DMAs (512 KiB per f-half) ride
    the gpsimd SWDGE ring. PSUM -> SBUF copies cast to bf16 and are split
    across the vector engine (f-half 0) and scalar engine (f-half 1).
  - The host casts the (F, 18432) bf16 shards up to fp32 and transposes back.

bf16 rounding of d, y and out adds ~3e-3 relative error (tolerance 1e-2).
"""

import sys

sys.path.insert(0, "/opt/trn_rl_repo")

from contextlib import ExitStack

import ml_dtypes
import numpy as np

import concourse.bacc as bacc
import concourse.mybir as mybir
import concourse.tile as tile
from concourse.bass_utils import run_bass_kernel_spmd

H, WG, F = 384, 384, 256
K = 256
NCORES = 8
HS = H // NCORES          # 48 grid rows per core
ROWS = HS * WG            # 18432 cells per core
BLK = 2048                # output columns per block (512 KiB bf16 per kc DMA)
NBLK = ROWS // BLK        # 9
NSB = BLK // 512          # 4 matmul sub-blocks of N=512 per block

F32 = mybir.dt.float32
BF16 = mybir.dt.bfloat16

_cache: dict = {}
last_results = None  # BassKernelResults of the most recent kernel() call


def _build_program():
    if "nc" in _cache:
        return _cache["nc"]

    nc = bacc.Bacc(
        "TRN2", target_bir_lowering=False, debug=False, num_devices=NCORES
    )

    dt_ext = nc.dram_tensor("d_t", [K, ROWS], BF16, kind="ExternalInput").ap()
    stT_ext = nc.dram_tensor("station_t", [K, K], F32, kind="ExternalInput").ap()
    w_ext = nc.dram_tensor("w_mat", [K, F], F32, kind="ExternalInput").ap()
    out_ext = nc.dram_tensor("out_t", [F, ROWS], BF16, kind="ExternalOutput").ap()

    with tile.TileContext(nc) as tc, ExitStack() as ctx:
        const = ctx.enter_context(tc.tile_pool(name="const", bufs=1))
        dpool = ctx.enter_context(tc.tile_pool(name="din", bufs=4))
        o0pool = ctx.enter_context(tc.tile_pool(name="dout0", bufs=3))
        o1pool = ctx.enter_context(tc.tile_pool(name="dout1", bufs=3))
        # 4 tags x 1 buf x [128, 1024] f32 = 2 banks each = all 8 PSUM banks
        mpsum = ctx.enter_context(tc.tile_pool(name="mpsum", bufs=1, space="PSUM"))

        # --- constants on the scalar HWDGE ring (sync ring is for d_t) ------
        stT = const.tile([128, 2, K], F32)
        nc.scalar.dma_start(
            stT[:, :, :], stT_ext.rearrange("(cc cp) k -> cp cc k", cc=2)
        )
        w_sb = const.tile([128, 2, F], F32)
        nc.scalar.dma_start(
            w_sb[:, :, :], w_ext.rearrange("(cc cp) f -> cp cc f", cc=2)
        )

        # --- y = station_feats @ W (precise fp32), k-major, cast to bf16 ----
        y_sb = const.tile([128, 2, F], BF16)
        for kc in range(2):
            yps = mpsum.tile([128, 1024], F32, name=f"yps{kc}", tag=f"p0{kc}")
            for cc in range(2):
                nc.tensor.matmul(
                    yps[:, :F],
                    stT[:, cc, kc * 128 : (kc + 1) * 128],
                    w_sb[:, cc, :],
                    start=(cc == 0),
                    stop=(cc == 1),
                )
            nc.vector.tensor_copy(y_sb[:, kc, :], yps[:, :F])

        # --- main loop: out^T[f, m] = sum_k y[k, f] * d_t[k, m] -------------
        for b in range(NBLK):
            m0 = b * BLK
            din = []
            for kc in range(2):
                dkc = dpool.tile([128, BLK], BF16, name=f"din{kc}", tag=f"din{kc}")
                nc.sync.dma_start(
                    dkc[:, :], dt_ext[kc * 128 : (kc + 1) * 128, m0 : m0 + BLK]
                )
                din.append(dkc)
            for fh in range(2):
                opool = o0pool if fh == 0 else o1pool
                dout = opool.tile([128, BLK], BF16, name=f"dout{fh}", tag=f"dout{fh}")
                ps = [
                    mpsum.tile([128, 1024], F32, name=f"p{fh}{h}", tag=f"p{fh}{h}")
                    for h in range(2)
                ]
                for kc in range(2):
                    for sb in range(NSB):
                        nc.tensor.matmul(
                            ps[sb // 2][:, (sb % 2) * 512 : (sb % 2 + 1) * 512],
                            y_sb[:, kc, fh * 128 : (fh + 1) * 128],
                            din[kc][:, sb * 512 : (sb + 1) * 512],
                            start=(kc == 0),
                            stop=(kc == 1),
                        )
                copy = nc.vector.tensor_copy if fh == 0 else nc.scalar.copy
                for h in range(2):
                    copy(dout[:, h * 1024 : (h + 1) * 1024], ps[h][:, :])
                nc.scalar.dma_start(
                    out_ext[fh * 128 : (fh + 1) * 128, m0 : m0 + BLK],
                    dout[:, :],
                )

    nc.compile()
    _cache["nc"] = nc
    return nc


def kernel(x, d, W, sx, sy):
    x = np.asarray(x, dtype=np.float32)
    d = np.asarray(d, dtype=np.float32)
    W = np.asarray(W, dtype=np.float32)
    sx = np.asarray(sx, dtype=np.int32)
    sy = np.asarray(sy, dtype=np.int32)

    # Host-side gather of the K station feature vectors (replicated to all
    # cores, per the sharding strategy), pre-transposed to contraction-major.
    station_t = np.ascontiguousarray(x[sx, sy].T)
    bf16 = ml_dtypes.bfloat16

    nc = _build_program()

    dd = d.reshape(NCORES, ROWS, K)
    in_maps = []
    for c in range(NCORES):
        in_maps.append(
            {
                "d_t": dd[c].T.astype(bf16),  # (K, ROWS) contraction-major
                "station_t": station_t,
                "w_mat": W,
            }
        )

    res = run_bass_kernel_spmd(nc, in_maps, list(range(NCORES)))
    global last_results
    last_results = res
    out = np.concatenate(
        [
            np.asarray(r["out_t"]).astype(np.float32).T.reshape(HS, WG, F)
            for r in res.results
        ],
        axis=0,
    )
    return out


if __name__ == "__main__":
    rng = np.random.default_rng(0)
    x = rng.standard_normal((H, WG, F), dtype=np.float32)
    d = rng.random((H, WG, K), dtype=np.float32)
    W = rng.standard_normal((K, F), dtype=np.float32) / np.sqrt(F)
    sx = rng.integers(0, H, size=(K,)).astype(np.int32)
    sy = rng.integers(0, WG, size=(K,)).astype(np.int32)
    out = kernel(x, d, W, sx, sy)
    y = x[sx, sy].astype(np.float64) @ W.astype(np.float64)
    exp = d.reshape(-1, K).astype(np.float64) @ y
    exp = exp.reshape(H, WG, F)
    err = np.linalg.norm(out - exp) / np.linalg.norm(exp)
    print("rel err:", err)


# revision 8
# speedup vs baseline: 1.9002x; 1.0981x over previous
"""Trainium2 Bass kernel for nn_ContextualViewModel_48833778155979.

Computation (see reference):
    station_feats = x[sx, sy]            # (K, F) gather -- on host (replicated)
    y = station_feats @ W                # (K, F) tiny matmul -- on device, fp32
    res[h, w, :] = sum_k d[h, w, k] * y[k, :]   # big (H*W, K) @ (K, F) matmul

Sharding: H axis split across 8 cores (48 rows each -> 18432 grid cells/core).

Device strategy per core (v5 -- transpose-free, bf16 I/O, double-buffered PSUM):
  - The host ships d pre-transposed per core as d_t = (K, 18432) bf16, so the
    contraction dim k is already on SBUF partitions: no on-device transposes.
  - y = (x[sx,sy]) @ W is computed once on device with precise fp32 matmuls
    and cast to bf16; its four 128x128 chunks are the stationary operands.
  - A short burst of throwaway matmuls right after the y stage keeps the PE
    HAM activity monitor busy so the array is un-throttled (2.4 GHz) by the
    time the first d_t block lands.
  - Main loop: per 2048-column block, two 1024-column half-blocks; per
    (half-block, f-half) four matmuls of N=512 accumulate over the two
    128-wide k chunks into a [128, 1024] fp32 PSUM tile (2 banks). Tiles are
    keyed by (f-half, half-block parity): 4 tags x 2 banks = all 8 banks,
    giving true double buffering -- a block's matmuls never wait on the
    previous block's PSUM evacuation.
  - PSUM -> SBUF copies cast to bf16: vector engine takes f-half 0, scalar
    engine f-half 1. One 1 MiB output DMA per block on the scalar HWDGE
    ring; input DMAs (256 KiB per k-chunk half-block) ride the sync HWDGE
    ring; the two constant DMAs ride the scalar ring up front.
  - The host casts the (F, 18432) bf16 shards up to fp32 and transposes back.

bf16 rounding of d, y and out adds ~3e-3 relative error (tolerance 1e-2).
"""

import sys

sys.path.insert(0, "/opt/trn_rl_repo")

from contextlib import ExitStack

import ml_dtypes
import numpy as np

import concourse.bacc as bacc
import concourse.mybir as mybir
import concourse.tile as tile
from concourse.bass_utils import run_bass_kernel_spmd

H, WG, F = 384, 384, 256
K = 256
NCORES = 8
HS = H // NCORES          # 48 grid rows per core
ROWS = HS * WG            # 18432 cells per core
BLK = 2048                # output columns per block
NBLK = ROWS // BLK        # 9
HB = 1024                 # half-block columns (one PSUM tile / input DMA)
WARMUP_MM = 20            # dummy N=256 matmuls to warm the PE HAM

F32 = mybir.dt.float32
BF16 = mybir.dt.bfloat16

_cache: dict = {}
last_results = None  # BassKernelResults of the most recent kernel() call


def _build_program():
    if "nc" in _cache:
        return _cache["nc"]

    nc = bacc.Bacc(
        "TRN2", target_bir_lowering=False, debug=False, num_devices=NCORES
    )

    dt_ext = nc.dram_tensor("d_t", [K, ROWS], BF16, kind="ExternalInput").ap()
    stT_ext = nc.dram_tensor("station_t", [K, K], F32, kind="ExternalInput").ap()
    w_ext = nc.dram_tensor("w_mat", [K, F], F32, kind="ExternalInput").ap()
    out_ext = nc.dram_tensor("out_t", [F, ROWS], BF16, kind="ExternalOutput").ap()

    with tile.TileContext(nc) as tc, ExitStack() as ctx:
        const = ctx.enter_context(tc.tile_pool(name="const", bufs=1))
        dpool = ctx.enter_context(tc.tile_pool(name="din", bufs=8))
        opool = ctx.enter_context(tc.tile_pool(name="dout", bufs=3))
        # 4 tags x 1 buf x [128, 1024] f32 = 2 banks each = all 8 PSUM banks
        mpsum = ctx.enter_context(tc.tile_pool(name="mpsum", bufs=1, space="PSUM"))

        # --- constants on the scalar HWDGE ring (sync ring is for d_t) ------
        stT = const.tile([128, 2, K], F32)
        nc.scalar.dma_start(
            stT[:, :, :], stT_ext.rearrange("(cc cp) k -> cp cc k", cc=2)
        )
        w_sb = const.tile([128, 2, F], F32)
        nc.scalar.dma_start(
            w_sb[:, :, :], w_ext.rearrange("(cc cp) f -> cp cc f", cc=2)
        )

        # --- y = station_feats @ W (precise fp32), k-major, cast to bf16 ----
        y_sb = const.tile([128, 2, F], BF16)
        for kc in range(2):
            yps = mpsum.tile([128, 1024], F32, name=f"yps{kc}", tag=f"q0{kc}")
            for cc in range(2):
                nc.tensor.matmul(
                    yps[:, :F],
                    stT[:, cc, kc * 128 : (kc + 1) * 128],
                    w_sb[:, cc, :],
                    start=(cc == 0),
                    stop=(cc == 1),
                )
            nc.vector.tensor_copy(y_sb[:, kc, :], yps[:, :F])

        # --- PE warm-up: keep the array busy while the first block loads ----
        wps = mpsum.tile([128, 1024], F32, name="wps", tag="q11")
        for _ in range(WARMUP_MM):
            nc.tensor.matmul(
                wps[:, :F],
                y_sb[:, 0, 0:128],
                y_sb[:, 1, :],
                start=True,
                stop=True,
            )

        # --- main loop: out^T[f, m] = sum_k y[k, f] * d_t[k, m] -------------
        for b in range(NBLK):
            m0 = b * BLK
            din = [[None, None], [None, None]]  # [kc][hb]
            for hb in range(2):
                for kc in range(2):
                    t = dpool.tile(
                        [128, HB], BF16, name=f"din{kc}", tag=f"din{kc}"
                    )
                    c0 = m0 + hb * HB
                    nc.sync.dma_start(
                        t[:, :], dt_ext[kc * 128 : (kc + 1) * 128, c0 : c0 + HB]
                    )
                    din[kc][hb] = t
            dout = opool.tile([128, 2, BLK], BF16, tag="dout")
            for hb in range(2):
                for fh in range(2):
                    q = mpsum.tile(
                        [128, 1024], F32, name=f"q{fh}{hb % 2}", tag=f"q{fh}{hb % 2}"
                    )
                    for kc in range(2):
                        for sb in range(2):
                            nc.tensor.matmul(
                                q[:, sb * 512 : (sb + 1) * 512],
                                y_sb[:, kc, fh * 128 : (fh + 1) * 128],
                                din[kc][hb][:, sb * 512 : (sb + 1) * 512],
                                start=(kc == 0),
                                stop=(kc == 1),
                            )
                    copy = nc.vector.tensor_copy if fh == 0 else nc.scalar.copy
                    copy(dout[:, fh, hb * HB : (hb + 1) * HB], q[:, :])
            nc.scalar.dma_start(
                out_ext[:, m0 : m0 + BLK].rearrange("(fc fp) m -> fp fc m", fc=2),
                dout[:, :, :],
            )

    nc.compile()
    _cache["nc"] = nc
    return nc


def kernel(x, d, W, sx, sy):
    x = np.asarray(x, dtype=np.float32)
    d = np.asarray(d, dtype=np.float32)
    W = np.asarray(W, dtype=np.float32)
    sx = np.asarray(sx, dtype=np.int32)
    sy = np.asarray(sy, dtype=np.int32)

    # Host-side gather of the K station feature vectors (replicated to all
    # cores, per the sharding strategy), pre-transposed to contraction-major.
    station_t = np.ascontiguousarray(x[sx, sy].T)
    bf16 = ml_dtypes.bfloat16

    nc = _build_program()

    dd = d.reshape(NCORES, ROWS, K)
    in_maps = []
    for c in range(NCORES):
        in_maps.append(
            {
                "d_t": dd[c].T.astype(bf16),  # (K, ROWS) contraction-major
                "station_t": station_t,
                "w_mat": W,
            }
        )

    res = run_bass_kernel_spmd(nc, in_maps, list(range(NCORES)))
    global last_results
    last_results = res
    out = np.concatenate(
        [
            np.asarray(r["out_t"]).astype(np.float32).T.reshape(HS, WG, F)
            for r in res.results
        ],
        axis=0,
    )
    return out


if __name__ == "__main__":
    rng = np.random.default_rng(0)
    x = rng.standard_normal((H, WG, F), dtype=np.float32)
    d = rng.random((H, WG, K), dtype=np.float32)
    W = rng.standard_normal((K, F), dtype=np.float32) / np.sqrt(F)
    sx = rng.integers(0, H, size=(K,)).astype(np.int32)
    sy = rng.integers(0, WG, size=(K,)).astype(np.int32)
    out = kernel(x, d, W, sx, sy)
    y = x[sx, sy].astype(np.float64) @ W.astype(np.float64)
    exp = d.reshape(-1, K).astype(np.float64) @ y
    exp = exp.reshape(H, WG, F)
    err = np.linalg.norm(out - exp) / np.linalg.norm(exp)
    print("rel err:", err)


# revision 9
# speedup vs baseline: 2.0603x; 1.0842x over previous
"""Trainium2 Bass kernel for nn_ContextualViewModel_48833778155979.

Computation (see reference):
    station_feats = x[sx, sy]            # (K, F) gather -- on host (replicated)
    y = station_feats @ W                # (K, F) tiny matmul -- on device, fp32
    res[h, w, :] = sum_k d[h, w, k] * y[k, :]   # big (H*W, K) @ (K, F) matmul

Sharding: H axis split across 8 cores (48 rows each -> 18432 grid cells/core).

Device strategy per core (v6 -- transpose-free, bf16 I/O, double-buffered
PSUM, fast startup):
  - The host ships d pre-transposed per core as d_t = (K, 18432) bf16, so the
    contraction dim k is already on SBUF partitions: no on-device transposes.
  - Startup: a burst of throwaway matmuls on memset scratch (no data
    dependencies) warms the PE HAM clock gate while the first DMAs fly.
    The two constant matrices arrive as ONE host-packed 512 KiB DMA laid out
    exactly as SBUF wants it (no rearrange, 4 KiB per partition line).
  - y = (x[sx,sy]) @ W is computed on device with precise fp32 matmuls and
    cast to bf16; its four 128x128 chunks are the stationary operands.
  - Main loop: per 2048-column block, two 1024-column half-blocks; per
    (half-block, f-half) four matmuls of N=512 accumulate over the two
    128-wide k chunks into a [128, 1024] fp32 PSUM tile (2 banks). Tiles are
    keyed by (f-half, half-block parity): 4 tags x 2 banks = all 8 banks,
    giving true double buffering.
  - PSUM -> SBUF copies cast to bf16: vector engine takes f-half 0, scalar
    engine f-half 1. One 512 KiB output DMA per half-block on the scalar
    HWDGE ring (earlier drain, shorter tail); input DMAs (256 KiB per
    k-chunk half-block) ride the sync HWDGE ring.
  - The host casts the (F, 18432) bf16 shards up to fp32 and transposes back.

bf16 rounding of d, y and out adds ~3e-3 relative error (tolerance 1e-2).
"""

import sys

sys.path.insert(0, "/opt/trn_rl_repo")

from contextlib import ExitStack

import ml_dtypes
import numpy as np

import concourse.bacc as bacc
import concourse.mybir as mybir
import concourse.tile as tile
from concourse.bass_utils import run_bass_kernel_spmd

H, WG, F = 384, 384, 256
K = 256
NCORES = 8
HS = H // NCORES          # 48 grid rows per core
ROWS = HS * WG            # 18432 cells per core
BLK = 2048                # output columns per block
NBLK = ROWS // BLK        # 9
HB = 1024                 # half-block columns (one PSUM tile / input DMA)
WARMUP_MM = 24            # dummy N=256 matmuls to warm the PE HAM

F32 = mybir.dt.float32
BF16 = mybir.dt.bfloat16

_cache: dict = {}
last_results = None  # BassKernelResults of the most recent kernel() call


def _build_program():
    if "nc" in _cache:
        return _cache["nc"]

    nc = bacc.Bacc(
        "TRN2", target_bir_lowering=False, debug=False, num_devices=NCORES
    )

    dt_ext = nc.dram_tensor("d_t", [K, ROWS], BF16, kind="ExternalInput").ap()
    # Host-packed constants: [cp, cc, 0, :] = station_t chunk, [cp, cc, 1, :] = W chunk
    cst_ext = nc.dram_tensor(
        "const_pack", [128, 2, 2, K], F32, kind="ExternalInput"
    ).ap()
    out_ext = nc.dram_tensor("out_t", [F, ROWS], BF16, kind="ExternalOutput").ap()

    with tile.TileContext(nc) as tc, ExitStack() as ctx:
        const = ctx.enter_context(tc.tile_pool(name="const", bufs=1))
        dpool = ctx.enter_context(tc.tile_pool(name="din", bufs=8))
        opool = ctx.enter_context(tc.tile_pool(name="dout", bufs=3))
        # 4 tags x 1 buf x [128, 1024] f32 = 2 banks each = all 8 PSUM banks
        mpsum = ctx.enter_context(tc.tile_pool(name="mpsum", bufs=1, space="PSUM"))

        # --- PE warm-up on memset scratch: zero data deps, starts at t=0 ----
        wa = const.tile([128, 128], BF16)
        wb = const.tile([128, 256], BF16)
        nc.vector.memset(wa[:, :], 0.0)
        nc.vector.memset(wb[:, :], 0.0)
        wps = mpsum.tile([128, 1024], F32, name="wps", tag="q11")
        for _ in range(WARMUP_MM):
            nc.tensor.matmul(wps[:, :F], wa[:, :], wb[:, :], start=True, stop=True)

        # --- constants: one packed 512 KiB DMA on the scalar HWDGE ring -----
        cst = const.tile([128, 2, 2, K], F32)
        nc.scalar.dma_start(cst[:, :, :, :], cst_ext)

        # --- y = station_feats @ W (precise fp32), k-major, cast to bf16 ----
        y_sb = const.tile([128, 2, F], BF16)
        for kc in range(2):
            yps = mpsum.tile([128, 1024], F32, name=f"yps{kc}", tag=f"q0{kc}")
            for cc in range(2):
                nc.tensor.matmul(
                    yps[:, :F],
                    cst[:, cc, 0, kc * 128 : (kc + 1) * 128],
                    cst[:, cc, 1, :],
                    start=(cc == 0),
                    stop=(cc == 1),
                )
            nc.vector.tensor_copy(y_sb[:, kc, :], yps[:, :F])

        # --- main loop: out^T[f, m] = sum_k y[k, f] * d_t[k, m] -------------
        for b in range(NBLK):
            m0 = b * BLK
            din = [[None, None], [None, None]]  # [kc][hb]
            for hb in range(2):
                for kc in range(2):
                    t = dpool.tile(
                        [128, HB], BF16, name=f"din{kc}", tag=f"din{kc}"
                    )
                    c0 = m0 + hb * HB
                    nc.sync.dma_start(
                        t[:, :], dt_ext[kc * 128 : (kc + 1) * 128, c0 : c0 + HB]
                    )
                    din[kc][hb] = t
            dout = opool.tile([128, 2, BLK], BF16, tag="dout")
            for hb in range(2):
                for fh in range(2):
                    q = mpsum.tile(
                        [128, 1024], F32, name=f"q{fh}{hb % 2}", tag=f"q{fh}{hb % 2}"
                    )
                    for kc in range(2):
                        for sb in range(2):
                            nc.tensor.matmul(
                                q[:, sb * 512 : (sb + 1) * 512],
                                y_sb[:, kc, fh * 128 : (fh + 1) * 128],
                                din[kc][hb][:, sb * 512 : (sb + 1) * 512],
                                start=(kc == 0),
                                stop=(kc == 1),
                            )
                    copy = nc.vector.tensor_copy if fh == 0 else nc.scalar.copy
                    copy(dout[:, fh, hb * HB : (hb + 1) * HB], q[:, :])
                c0 = m0 + hb * HB
                nc.scalar.dma_start(
                    out_ext[:, c0 : c0 + HB].rearrange("(fc fp) m -> fp fc m", fc=2),
                    dout[:, :, hb * HB : (hb + 1) * HB],
                )

    nc.compile()
    _cache["nc"] = nc
    return nc


def kernel(x, d, W, sx, sy):
    x = np.asarray(x, dtype=np.float32)
    d = np.asarray(d, dtype=np.float32)
    W = np.asarray(W, dtype=np.float32)
    sx = np.asarray(sx, dtype=np.int32)
    sy = np.asarray(sy, dtype=np.int32)

    # Host-side gather of the K station feature vectors (replicated to all
    # cores, per the sharding strategy), pre-transposed to contraction-major,
    # packed together with W in the exact SBUF layout (one DMA, no rearrange).
    station_t = x[sx, sy].T                      # (c, k)
    cst = np.empty((128, 2, 2, K), dtype=np.float32)
    for cc in range(2):
        cst[:, cc, 0, :] = station_t[cc * 128 : (cc + 1) * 128, :]
        cst[:, cc, 1, :] = W[cc * 128 : (cc + 1) * 128, :]
    bf16 = ml_dtypes.bfloat16

    nc = _build_program()

    dd = d.reshape(NCORES, ROWS, K)
    in_maps = []
    for c in range(NCORES):
        in_maps.append(
            {
                "d_t": dd[c].T.astype(bf16),  # (K, ROWS) contraction-major
                "const_pack": cst,
            }
        )

    res = run_bass_kernel_spmd(nc, in_maps, list(range(NCORES)))
    global last_results
    last_results = res
    out = np.concatenate(
        [
            np.asarray(r["out_t"]).astype(np.float32).T.reshape(HS, WG, F)
            for r in res.results
        ],
        axis=0,
    )
    return out


if __name__ == "__main__":
    rng = np.random.default_rng(0)
    x = rng.standard_normal((H, WG, F), dtype=np.float32)
    d = rng.random((H, WG, K), dtype=np.float32)
    W = rng.standard_normal((K, F), dtype=np.float32) / np.sqrt(F)
    sx = rng.integers(0, H, size=(K,)).astype(np.int32)
    sy = rng.integers(0, WG, size=(K,)).astype(np.int32)
    out = kernel(x, d, W, sx, sy)
    y = x[sx, sy].astype(np.float64) @ W.astype(np.float64)
    exp = d.reshape(-1, K).astype(np.float64) @ y
    exp = exp.reshape(H, WG, F)
    err = np.linalg.norm(out - exp) / np.linalg.norm(exp)
    print("rel err:", err)
